# revision 4
# baseline (speedup 1.0000x reference)
"""Multi-head attention kernel for Trainium2, SPMD across 8 NeuronCores.

Problem: x[8,16,256,384] -> attention(8 heads, head_dim 64) -> [8,16,256,384]
Sharding: data-parallel over batch b (1 batch element per core, weights
replicated). Each core processes 16 independent slices of [256 tokens, 384].

Per-slice dataflow (all activations kept feature-major, i.e. transposed):
  xT[d,t]   = PE-transpose of x slice               [384, 256]
  qkT[e,t]  = w_qkv[:,e].T @ xT  (e in 0..1023)     q^T,k^T feature-major
  v[t,e]    = xT.T @ w_qkv[:, 1024:1536]            natural layout
  sT[j,i]   = k_h^T.T @ q_h^T   (per head, K=64)    scores transposed
  pT        = exp(sT / 8)
  o[0:64]   = v_h.T @ pT        (AV, accumulated over j-chunks)
  o[64:65]  = ones.T @ pT       (softmax denominator via matmul)
  oT        = o[0:64] * broadcast(1/o[64])          normalized, feature-major
  out[t,:]  = oT.T @ w_out + b_out
"""

import sys
import types

sys.path.insert(0, "/opt/trn_rl_repo")

import numpy as np

import concourse.bass as bass
import concourse.bacc as bacc
import concourse.mybir as mybir
import concourse.tile as tile
from concourse.bass_utils import run_bass_kernel_spmd

N_CORES = 8
B, P, N, D = 8, 16, 256, 384
H, HD = 8, 64
INNER = H * HD  # 512
SCALE = HD ** -0.5
F32 = mybir.dt.float32

# "f32r" uses the fast fp32 PE mode (1 cycle/row at N>=256); "f32" is the
# exact 4-cycle/row mode.
MM_MODE = "f32r"


def _register_ntff_hook():
    """Make trace=True work under axon when antenv.axon_hooks is absent."""
    if "antenv.axon_hooks" in sys.modules:
        return
    try:
        from trn_agent_boot.trn_boot import _ntff_profile_via_ctypes
    except ImportError:
        return
    hook = _ntff_profile_via_ctypes("/opt/axon/libaxon_pjrt.so")
    mod = types.ModuleType("antenv.axon_hooks")
    mod.get_axon_ntff_profile_hook = lambda: hook
    sys.modules["antenv.axon_hooks"] = mod


def build(mm_mode=MM_MODE):
    nc = bacc.Bacc("TRN2", target_bir_lowering=False, debug=False,
                   num_devices=N_CORES)
    MDT = mybir.dt.float32r if mm_mode == "f32r" else F32
    x_ext = nc.declare_dram_parameter("x", [P, N, D], F32, isOutput=False)
    wq_ext = nc.declare_dram_parameter("w_qkv", [D, 3 * INNER], MDT,
                                       isOutput=False)
    wo_ext = nc.declare_dram_parameter("w_out", [INNER, D], MDT,
                                       isOutput=False)
    bo_ext = nc.declare_dram_parameter("b_out", [D], F32, isOutput=False)
    id_ext = nc.declare_dram_parameter("ident", [128, 128], F32,
                                       isOutput=False)
    out_ext = nc.declare_dram_parameter("out", [P, N, D], F32, isOutput=True)

    def mm(ap):
        return ap

    Exp = mybir.ActivationFunctionType.Exp

    with tile.TileContext(nc) as tc:
        with (
            tc.tile_pool(name="const", bufs=1) as const,
            tc.tile_pool(name="xn", bufs=2) as xn_pool,
            tc.tile_pool(name="xt", bufs=2) as xt_pool,
            tc.tile_pool(name="qk", bufs=2) as qk_pool,
            tc.tile_pool(name="vp", bufs=2) as v_pool,
            tc.tile_pool(name="pt", bufs=4) as p_pool,
            tc.tile_pool(name="ot", bufs=2) as ot_pool,
            tc.tile_pool(name="ob", bufs=3) as ob_pool,
            tc.tile_pool(name="rs", bufs=4) as rs_pool,
            tc.tile_pool(name="bc", bufs=4) as bc_pool,
            tc.tile_pool(name="mmps", bufs=3, space="PSUM") as mm_ps,
            tc.tile_pool(name="sps", bufs=3, space="PSUM") as s_ps,
            tc.tile_pool(name="ops", bufs=2, space="PSUM") as o_ps,
        ):
            # ---- constants (loaded once) ----
            w_sb = const.tile([128, 3 * 1536], MDT, tag="w_sb")
            for kc in range(3):
                nc.sync.dma_start(w_sb[:, kc * 1536:(kc + 1) * 1536],
                                  wq_ext.ap()[kc * 128:(kc + 1) * 128, :])
            wo_sb = const.tile([128, 4 * 384], MDT, tag="wo_sb")
            for kc in range(4):
                nc.sync.dma_start(wo_sb[:, kc * 384:(kc + 1) * 384],
                                  wo_ext.ap()[kc * 128:(kc + 1) * 128, :])
            id_sb = const.tile([128, 128], F32, tag="id_sb")
            nc.sync.dma_start(id_sb[:], id_ext.ap())
            bt_sb = const.tile([1, 384], F32, tag="bt_sb")
            nc.sync.dma_start(bt_sb[:], bo_ext.ap().unsqueeze(0))
            bias_sb = const.tile([128, 384], F32, tag="bias_sb")
            nc.gpsimd.partition_broadcast(bias_sb[:], bt_sb[0:1, :])

            for s in range(P):
                # ---- load x slice and transpose to xT [384, 256] ----
                xn = xn_pool.tile([128, 2 * 384], F32, tag="xn")
                for t in range(2):
                    nc.sync.dma_start(xn[:, t * 384:(t + 1) * 384],
                                      x_ext.ap()[s, t * 128:(t + 1) * 128, :])
                xt = xt_pool.tile([128, 3 * 256], MDT, tag="xt")
                for t in range(2):
                    for kc in range(3):
                        tp = mm_ps.tile([128, 512], F32, tag="mmps")
                        nc.tensor.transpose(
                            tp[:, 0:128],
                            xn[:, t * 384 + kc * 128: t * 384 + (kc + 1) * 128],
                            id_sb[:])
                        nc.vector.tensor_copy(
                            xt[:, kc * 256 + t * 128: kc * 256 + (t + 1) * 128],
                            tp[:, 0:128])

                # ---- qkT chunks (features m*128..m*128+127, m<8 = q,k) ----
                qk = qk_pool.tile([128, 8 * 256], MDT, tag="qk")
                for m in range(8):
                    ps = mm_ps.tile([128, 512], F32, tag="mmps")
                    for kc in range(3):
                        nc.tensor.matmul(
                            ps[:, 0:256],
                            mm(w_sb[:, kc * 1536 + m * 128:
                                    kc * 1536 + (m + 1) * 128]),
                            mm(xt[:, kc * 256:(kc + 1) * 256]),
                            start=(kc == 0), stop=(kc == 2))
                    nc.scalar.copy(qk[:, m * 256:(m + 1) * 256], ps[:, 0:256])

                # ---- v natural [token, 2*520]: per chunk 8 x (64 v | 1 ones) ----
                v = v_pool.tile([128, 2 * 520], MDT, tag="v")
                ones_cols = v[:].rearrange("p (a c) -> p a c", c=65)[:, :, 64:65]
                nc.gpsimd.memset(ones_cols.bitcast(F32), 1.0)
                for t in range(2):
                    ps = mm_ps.tile([128, 512], F32, tag="mmps")
                    for kc in range(3):
                        nc.tensor.matmul(
                            ps[:],
                            mm(xt[:, kc * 256 + t * 128:
                                   kc * 256 + (t + 1) * 128]),
                            mm(w_sb[:, kc * 1536 + 1024: kc * 1536 + 1536]),
                            start=(kc == 0), stop=(kc == 2))
                    for h in range(8):
                        nc.vector.tensor_copy(
                            v[:, t * 520 + h * 65: t * 520 + h * 65 + 64],
                            ps[:, h * 64:(h + 1) * 64])

                # ---- attention per head ----
                ot = ot_pool.tile([128, 4 * 256], MDT, tag="ot")
                for h in range(8):
                    po = (h % 2) * 64
                    qc = h // 2
                    kc_ = 4 + h // 2
                    pts = []
                    for jc in range(2):
                        sps = s_ps.tile([128, 256], F32, tag="sps")
                        nc.tensor.matmul(
                            sps[:],
                            mm(qk[po:po + 64,
                                  kc_ * 256 + jc * 128:
                                  kc_ * 256 + (jc + 1) * 128]),
                            mm(qk[po:po + 64, qc * 256:(qc + 1) * 256]),
                            start=True, stop=True,
                            tile_position=(po, 0))
                        pt = p_pool.tile([128, 256], MDT, tag="pt")
                        nc.scalar.activation(pt[:], sps[:], Exp, scale=SCALE)
                        pts.append(pt)
                    ops = o_ps.tile([128, 256], F32, tag="ops")
                    for jc in range(2):
                        nc.tensor.matmul(
                            ops[0:65, :],
                            mm(v[:, jc * 520 + h * 65: jc * 520 + h * 65 + 65]),
                            mm(pts[jc][:]),
                            start=(jc == 0), stop=(jc == 1))
                    rs = rs_pool.tile([1, 256], F32, tag="rs")
                    nc.vector.reciprocal(rs[:], ops[64:65, :])
                    bc = bc_pool.tile([64, 256], F32, tag="bc")
                    nc.gpsimd.partition_broadcast(bc[:], rs[0:1, :])
                    nc.vector.tensor_mul(
                        ot[po:po + 64, qc * 256:(qc + 1) * 256],
                        ops[0:64, :], bc[:])

                # ---- output projection + bias ----
                for t in range(2):
                    fps = mm_ps.tile([128, 512], F32, tag="mmps")
                    for kc in range(4):
                        nc.tensor.matmul(
                            fps[:, 0:384],
                            mm(ot[:, kc * 256 + t * 128:
                                   kc * 256 + (t + 1) * 128]),
                            mm(wo_sb[:, kc * 384:(kc + 1) * 384]),
                            start=(kc == 0), stop=(kc == 3))
                    ob = ob_pool.tile([128, 384], F32, tag="ob")
                    nc.vector.tensor_add(ob[:], fps[:, 0:384], bias_sb[:])
                    nc.sync.dma_start(out_ext.ap()[s, t * 128:(t + 1) * 128, :],
                                      ob[:])
    nc.compile()
    return nc


_CACHE = {}


def _get_nc(mm_mode=MM_MODE):
    if mm_mode not in _CACHE:
        _CACHE[mm_mode] = build(mm_mode)
    return _CACHE[mm_mode]


def _in_maps(inputs):
    x = np.ascontiguousarray(inputs["x"], dtype=np.float32)
    w_qkv = np.ascontiguousarray(inputs["w_qkv"], dtype=np.float32)
    w_out = np.ascontiguousarray(inputs["w_out"], dtype=np.float32)
    b_out = np.ascontiguousarray(inputs["b_out"], dtype=np.float32)
    ident = np.eye(128, dtype=np.float32)
    return [
        {"x": np.ascontiguousarray(x[i]), "w_qkv": w_qkv, "w_out": w_out,
         "b_out": b_out, "ident": ident}
        for i in range(N_CORES)
    ]


def run(inputs, trace=False, mm_mode=MM_MODE):
    """Returns (output [8,16,256,384], exec_time_ns or None)."""
    if trace:
        _register_ntff_hook()
    nc = _get_nc(mm_mode)
    res = run_bass_kernel_spmd(nc, _in_maps(inputs),
                               core_ids=list(range(N_CORES)), trace=trace)
    out = np.stack([res.results[i]["out"] for i in range(N_CORES)], axis=0)
    return out, res.exec_time_ns


def kernel(**inputs) -> np.ndarray:
    out, _ = run(inputs, trace=False)
    return out


# revision 7
# speedup vs baseline: 1.1558x; 1.1558x over previous
"""Multi-head attention kernel for Trainium2, SPMD across 8 NeuronCores.

Problem: x[8,16,256,384] -> attention(8 heads, head_dim 64) -> [8,16,256,384]
Sharding: data-parallel over batch b (1 batch element per core, weights
replicated). Each core processes 16 independent slices of [256 tokens, 384],
handled in pairs ("superslices") so the QKV matmuls stream N=512.

Per-slice dataflow (activations kept feature-major, i.e. transposed):
  xT[d,t]   = PE-transpose of x slice               [384, 256]
  qkT[e,t]  = w_qkv[:,e].T @ xT  (e in 0..1023)     q^T,k^T feature-major
  v[t,e]    = xT.T @ w_qkv[:, 1024:1536]            natural, 65-col head
              blocks whose last column is ones (fused softmax denominator)
  sT[j,i]   = k_h^T.T @ q_h^T   (per head, K=64; head pairs run row-tiled
              concurrently on the PE and share one [128,512] PSUM tile)
  pT        = exp(sT / 8)                           one ACT op per head pair
  o[0:65]   = [v_h | 1].T @ pT                      AV + denominator in one
  oT        = o[0:64] * broadcast(1/o[64])          normalized
  out[t,:]  = oT.T @ w_out + b_out
"""

import sys
import types

sys.path.insert(0, "/opt/trn_rl_repo")

import numpy as np

import concourse.bass as bass
import concourse.bacc as bacc
import concourse.mybir as mybir
import concourse.tile as tile
from concourse.bass_utils import run_bass_kernel_spmd

N_CORES = 8
B, P, N, D = 8, 16, 256, 384
H, HD = 8, 64
INNER = H * HD  # 512
SCALE = HD ** -0.5
F32 = mybir.dt.float32

MM_MODE = "bf16"  # "bf16" | "f32r" | "f32"


def _mdt(mm_mode):
    return {"bf16": mybir.dt.bfloat16,
            "f32r": mybir.dt.float32r,
            "f32": F32}[mm_mode]


def _np_mdt(mm_mode):
    if mm_mode == "bf16":
        import ml_dtypes
        return ml_dtypes.bfloat16
    return np.float32


def _register_ntff_hook():
    """Make trace=True work under axon when antenv.axon_hooks is absent."""
    if "antenv.axon_hooks" in sys.modules:
        return
    try:
        from trn_agent_boot.trn_boot import _ntff_profile_via_ctypes
    except ImportError:
        return
    hook = _ntff_profile_via_ctypes("/opt/axon/libaxon_pjrt.so")
    mod = types.ModuleType("antenv.axon_hooks")
    mod.get_axon_ntff_profile_hook = lambda: hook
    sys.modules["antenv.axon_hooks"] = mod


def build(mm_mode=MM_MODE):
    nc = bacc.Bacc("TRN2", target_bir_lowering=False, debug=False,
                   num_devices=N_CORES)
    MDT = _mdt(mm_mode)
    x_ext = nc.declare_dram_parameter("x", [P, N, D], MDT, isOutput=False)
    wq_ext = nc.declare_dram_parameter("w_qkv", [D, 3 * INNER], MDT,
                                       isOutput=False)
    wo_ext = nc.declare_dram_parameter("w_out", [INNER, D], MDT,
                                       isOutput=False)
    bo_ext = nc.declare_dram_parameter("b_out", [D], F32, isOutput=False)
    id_ext = nc.declare_dram_parameter("ident", [128, 128], MDT,
                                       isOutput=False)
    out_ext = nc.declare_dram_parameter("out", [P, N, D], F32, isOutput=True)

    Exp = mybir.ActivationFunctionType.Exp
    memset_dt = F32 if mm_mode != "bf16" else MDT

    with tile.TileContext(nc) as tc:
        with (
            tc.tile_pool(name="const", bufs=1) as const,
            tc.tile_pool(name="xn", bufs=2) as xn_pool,
            tc.tile_pool(name="xt", bufs=2) as xt_pool,
            tc.tile_pool(name="qk", bufs=2) as qk_pool,
            tc.tile_pool(name="vp", bufs=2) as v_pool,
            tc.tile_pool(name="pt", bufs=6) as p_pool,
            tc.tile_pool(name="ot", bufs=3) as ot_pool,
            tc.tile_pool(name="ob", bufs=3) as ob_pool,
            tc.tile_pool(name="rs", bufs=6) as rs_pool,
            tc.tile_pool(name="bc", bufs=6) as bc_pool,
            tc.tile_pool(name="mmps", bufs=3, space="PSUM") as mm_ps,
            tc.tile_pool(name="sps", bufs=3, space="PSUM") as s_ps,
            tc.tile_pool(name="ops", bufs=2, space="PSUM") as o_ps,
        ):
            # ---- constants (loaded once) ----
            w_sb = const.tile([128, 3 * 1536], MDT, tag="w_sb")
            for kc in range(3):
                nc.sync.dma_start(w_sb[:, kc * 1536:(kc + 1) * 1536],
                                  wq_ext.ap()[kc * 128:(kc + 1) * 128, :])
            wo_sb = const.tile([128, 4 * 384], MDT, tag="wo_sb")
            for kc in range(4):
                nc.sync.dma_start(wo_sb[:, kc * 384:(kc + 1) * 384],
                                  wo_ext.ap()[kc * 128:(kc + 1) * 128, :])
            id_sb = const.tile([128, 128], MDT, tag="id_sb")
            nc.sync.dma_start(id_sb[:], id_ext.ap())
            bt_sb = const.tile([1, 384], F32, tag="bt_sb")
            nc.sync.dma_start(bt_sb[:], bo_ext.ap().unsqueeze(0))
            bias_sb = const.tile([128, 384], F32, tag="bias_sb")
            nc.gpsimd.partition_broadcast(bias_sb[:], bt_sb[0:1, :])

            # m-chunk order: interleave q and k chunks so head-pair c has
            # its q (m=c) and k (m=4+c) chunks available early.
            m_order = [0, 4, 1, 5, 2, 6, 3, 7]

            for u in range(P // 2):  # superslice of 2 token slices
                # ---- load x slices ----
                xn = xn_pool.tile([128, 4 * 384], MDT, tag="xn")
                for a in range(2):
                    for t in range(2):
                        nc.sync.dma_start(
                            xn[:, (a * 2 + t) * 384:(a * 2 + t + 1) * 384],
                            x_ext.ap()[2 * u + a,
                                       t * 128:(t + 1) * 128, :])

                # ---- transpose to xT: kc block = [sliceA 256 | sliceB 256]
                xt = xt_pool.tile([128, 3 * 512], MDT, tag="xt")
                for kc in range(3):
                    tp = mm_ps.tile([128, 512], MDT, tag="mmps")
                    for at in range(4):
                        nc.tensor.transpose(
                            tp[:, at * 128:(at + 1) * 128],
                            xn[:, at * 384 + kc * 128: at * 384 + (kc + 1) * 128],
                            id_sb[:])
                    nc.vector.tensor_copy(xt[:, kc * 512:(kc + 1) * 512],
                                          tp[:])

                # ---- qkT chunks m (features m*128..m*128+127) ----
                qk = qk_pool.tile([128, 8 * 512], MDT, tag="qk")
                for mi, m in enumerate(m_order):
                    ps = mm_ps.tile([128, 512], F32, tag="mmps")
                    for kc in range(3):
                        nc.tensor.matmul(
                            ps[:],
                            w_sb[:, kc * 1536 + m * 128:
                                 kc * 1536 + (m + 1) * 128],
                            xt[:, kc * 512:(kc + 1) * 512],
                            start=(kc == 0), stop=(kc == 2))
                    if mi % 2 == 0:
                        nc.scalar.copy(qk[:, m * 512:(m + 1) * 512], ps[:])
                    else:
                        nc.vector.tensor_copy(qk[:, m * 512:(m + 1) * 512],
                                              ps[:])

                # ---- v: per (slice,tok-chunk) 520 cols: 8 x (64 v | one) ----
                v = v_pool.tile([128, 4 * 520], MDT, tag="v")
                ones_cols = v[:].rearrange("p (a c) -> p a c", c=65)[:, :, 64:65]
                nc.gpsimd.memset(ones_cols.bitcast(memset_dt), 1.0)
                for a in range(2):
                    for t in range(2):
                        ps = mm_ps.tile([128, 512], F32, tag="mmps")
                        for kc in range(3):
                            nc.tensor.matmul(
                                ps[:],
                                xt[:, kc * 512 + a * 256 + t * 128:
                                   kc * 512 + a * 256 + (t + 1) * 128],
                                w_sb[:, kc * 1536 + 1024: kc * 1536 + 1536],
                                start=(kc == 0), stop=(kc == 2))
                        dst = v[:, (a * 2 + t) * 520:(a * 2 + t) * 520 + 520]
                        dst = dst.rearrange("p (h c) -> p h c", c=65)[:, :, 0:64]
                        nc.vector.tensor_copy(
                            dst, ps[:].rearrange("p (h c) -> p h c", c=64))

                # ---- attention: head pairs (2c, 2c+1) per slice ----
                for a in range(2):
                    ot = ot_pool.tile([128, 4 * 256], MDT, tag="ot")
                    for c in range(4):
                        pts = []
                        for jc in range(2):
                            pt = p_pool.tile([128, 512], MDT, tag="pt")
                            for e in range(2):
                                sps = s_ps.tile([128, 256], F32, tag="sps")
                                nc.tensor.matmul(
                                    sps[:],
                                    qk[e * 64:e * 64 + 64,
                                       (4 + c) * 512 + a * 256 + jc * 128:
                                       (4 + c) * 512 + a * 256 + (jc + 1) * 128],
                                    qk[e * 64:e * 64 + 64,
                                       c * 512 + a * 256: c * 512 + (a + 1) * 256],
                                    start=True, stop=True,
                                    tile_position=(e * 64, 0))
                                nc.scalar.activation(
                                    pt[:, e * 256:(e + 1) * 256], sps[:], Exp,
                                    scale=SCALE)
                            pts.append(pt)
                        for e in range(2):
                            h = 2 * c + e
                            ops = o_ps.tile([128, 256], F32, tag="ops")
                            for jc in range(2):
                                nc.tensor.matmul(
                                    ops[0:65, :],
                                    v[:, (a * 2 + jc) * 520 + h * 65:
                                      (a * 2 + jc) * 520 + h * 65 + 65],
                                    pts[jc][:, e * 256:(e + 1) * 256],
                                    start=(jc == 0), stop=(jc == 1))
                            rs = rs_pool.tile([1, 256], F32, tag="rs")
                            nc.vector.reciprocal(rs[:], ops[64:65, :])
                            bc = bc_pool.tile([64, 256], F32, tag="bc")
                            nc.gpsimd.partition_broadcast(bc[:], rs[0:1, :])
                            nc.vector.tensor_mul(
                                ot[e * 64:(e + 1) * 64, c * 256:(c + 1) * 256],
                                ops[0:64, :], bc[:])

                    # ---- output projection + bias for slice a ----
                    for t in range(2):
                        fps = mm_ps.tile([128, 512], F32, tag="mmps")
                        for kc in range(4):
                            nc.tensor.matmul(
                                fps[:, 0:384],
                                ot[:, kc * 256 + t * 128:
                                   kc * 256 + (t + 1) * 128],
                                wo_sb[:, kc * 384:(kc + 1) * 384],
                                start=(kc == 0), stop=(kc == 3))
                        ob = ob_pool.tile([128, 384], F32, tag="ob")
                        nc.vector.tensor_add(ob[:], fps[:, 0:384], bias_sb[:])
                        nc.sync.dma_start(
                            out_ext.ap()[2 * u + a, t * 128:(t + 1) * 128, :],
                            ob[:])
    nc.compile()
    return nc


_CACHE = {}


def _get_nc(mm_mode=MM_MODE):
    if mm_mode not in _CACHE:
        _CACHE[mm_mode] = build(mm_mode)
    return _CACHE[mm_mode]


def _in_maps(inputs, mm_mode=MM_MODE):
    ndt = _np_mdt(mm_mode)
    x = np.asarray(inputs["x"]).astype(ndt)
    w_qkv = np.asarray(inputs["w_qkv"]).astype(ndt)
    w_out = np.asarray(inputs["w_out"]).astype(ndt)
    b_out = np.asarray(inputs["b_out"]).astype(np.float32)
    ident = np.eye(128).astype(ndt)
    return [
        {"x": np.ascontiguousarray(x[i]), "w_qkv": w_qkv, "w_out": w_out,
         "b_out": b_out, "ident": ident}
        for i in range(N_CORES)
    ]


def run(inputs, trace=False, mm_mode=MM_MODE):
    """Returns (output [8,16,256,384], exec_time_ns or None)."""
    if trace:
        _register_ntff_hook()
    nc = _get_nc(mm_mode)
    res = run_bass_kernel_spmd(nc, _in_maps(inputs, mm_mode),
                               core_ids=list(range(N_CORES)), trace=trace)
    out = np.stack([res.results[i]["out"] for i in range(N_CORES)], axis=0)
    return out, res.exec_time_ns


def kernel(**inputs) -> np.ndarray:
    out, _ = run(inputs, trace=False)
    return out
